# revision 6
# baseline (speedup 1.0000x reference)
"""Trainium2 Bass kernel: NestedTensorBlock (ViT block w/ LayerScale) over two
ragged groups  x1:[8,1024,1024]  x2:[8,512,1024],  D=1024, H=16 heads.

Sharding: pure data parallel — core c gets sequence c of each group
(1024 + 512 = 1536 tokens per core), no collectives.

Numerics: residual stream fp32; all GEMMs stream bf16 with fp32 PSUM
accumulation.  LayerScale gamma (1e-5) damps the attention/MLP branches, so
bf16 branch error is invisible in the output; host-side we fold LN affine,
the 1/sqrt(hd) score scale, and gamma into the weights.

Layout strategy per core (avoids all transposes except LN outputs):
  - LN token-major -> PE-transpose to xn^T [D, T]
  - q^T, k^T feature-major [D, T];  v token-major [T, H, 64+1] with a ones
    column (PV matmul then also emits the softmax denominator for free)
  - scores^T [k, q] straight from PE; exp on ACT with no max subtraction
    (scores are provably < ~3 for this problem's scales); P^T streams into
    the PV matmul with no transpose of probabilities
  - attn^T [D, T] feature-major feeds proj; proj/fc2 emit token-major for
    fp32 residual adds; fc1 emits feature-major h^T for gelu + fc2.
"""

import numpy as np

P = 128
HD = 64
LN_EPS = 1e-5
QG = 512  # token-group size for matmul free dims

FULL_CFG = dict(D=1024, H=16, SA=1024, SB=512, DF=4096)


def _ceil_div(a, b):
    return (a + b - 1) // b


def build_bass(cfg, has_row_bias=False):
    """Builds and compiles the per-core Bass program. Returns nc."""
    import concourse.bacc as bacc
    import concourse.tile as tile
    from concourse import mybir
    from contextlib import ExitStack

    f32 = mybir.dt.float32
    bf16 = mybir.dt.bfloat16
    AF = mybir.ActivationFunctionType
    OP = mybir.AluOpType

    D, H, SA, SB, DF = cfg["D"], cfg["H"], cfg["SA"], cfg["SB"], cfg["DF"]
    T = SA + SB
    NT = T // P                     # token tiles
    DC = D // P                     # D chunks (contraction chunks)
    FC = DF // P                    # fc1 output chunks
    HPC = P // HD                   # heads per feature chunk (2)
    n_og = _ceil_div(D, QG)         # output col groups for proj/fc2
    ogs = min(QG, D)
    n_tg = _ceil_div(T, QG)         # token groups for qkv
    seqs = [(0, SA // P), (SA // P, SB // P)]  # (tile offset, n tiles)
    max_kt = max(n for _, n in seqs)
    max_qg = min(QG, max(SA, SB))
    n_stat = _ceil_div(D, 512)

    nc = bacc.Bacc("TRN2", target_bir_lowering=False, debug=False)

    x_d = nc.dram_tensor("x", [T, D], f32, kind="ExternalInput").ap()
    wqkv_d = nc.dram_tensor("wqkv", [D, 3 * D], bf16, kind="ExternalInput").ap()
    wproj_d = nc.dram_tensor("wproj", [D, D], bf16, kind="ExternalInput").ap()
    wfc1_d = nc.dram_tensor("wfc1", [D, DF], bf16, kind="ExternalInput").ap()
    wfc2_d = nc.dram_tensor("wfc2", [DF, D], bf16, kind="ExternalInput").ap()
    ident_d = nc.dram_tensor("ident", [P, P], bf16, kind="ExternalInput").ap()
    bq_d = nc.dram_tensor("bq", [P, DC], f32, kind="ExternalInput").ap()
    bfc1_d = nc.dram_tensor("bfc1", [P, FC], f32, kind="ExternalInput").ap()
    if has_row_bias:
        bp_d = nc.dram_tensor("bprow", [1, D], f32, kind="ExternalInput").ap()
        bf2_d = nc.dram_tensor("bf2row", [1, D], f32, kind="ExternalInput").ap()
    y_d = nc.dram_tensor("y", [T, D], f32, kind="ExternalOutput").ap()

    with tile.TileContext(nc) as tc, ExitStack() as ctx:
        const = ctx.enter_context(tc.tile_pool(name="const", bufs=1))
        ident = const.tile([P, P], bf16)
        nc.sync.dma_start(ident[:], ident_d)
        bq_sb = const.tile([P, DC], f32)
        nc.sync.dma_start(bq_sb[:], bq_d)
        bfc1_sb = const.tile([P, FC], f32)
        nc.sync.dma_start(bfc1_sb[:], bfc1_d)
        eps_sb = const.tile([P, 1], f32)
        nc.vector.memset(eps_sb[:], LN_EPS)
        if has_row_bias:
            bp_row = const.tile([1, D], f32)
            nc.sync.dma_start(bp_row[:], bp_d)
            bp_b = const.tile([P, D], f32)
            nc.gpsimd.partition_broadcast(bp_b[:], bp_row[:])
            bf2_row = const.tile([1, D], f32)
            nc.sync.dma_start(bf2_row[:], bf2_d)
            bf2_b = const.tile([P, D], f32)
            nc.gpsimd.partition_broadcast(bf2_b[:], bf2_row[:])

        # small pools shared by both layernorms
        pst = ctx.enter_context(tc.tile_pool(name="st", bufs=4))
        pxn = ctx.enter_context(tc.tile_pool(name="xn", bufs=2))

        def layernorm_tile(xt):
            """xt: [P, D] f32 SBUF -> returns [P, D] bf16 standardized."""
            stats = pst.tile([P, n_stat, 6], f32, tag="stats")
            for i in range(n_stat):
                lo, hi = 512 * i, min(512 * (i + 1), D)
                nc.vector.bn_stats(stats[:, i, :], xt[:, lo:hi])
            mv = pst.tile([P, 2], f32, tag="mv")
            nc.vector.bn_aggr(mv[:], stats[:])
            std = pst.tile([P, 1], f32, tag="std")
            nc.scalar.activation(std[:], mv[:, 1:2], AF.Sqrt, bias=eps_sb[:])
            rstd = pst.tile([P, 1], f32, tag="rstd")
            nc.vector.reciprocal(rstd[:], std[:])
            xn = pxn.tile([P, D], bf16, tag="xn")
            nc.vector.tensor_scalar(
                xn[:], xt[:], mv[:, 0:1], rstd[:], op0=OP.subtract, op1=OP.mult
            )
            return xn

        # per-tile fp32 x2 scratch in DRAM (residual stream between halves)
        pdram = ctx.enter_context(tc.tile_pool(name="x2d", bufs=NT, space="DRAM"))
        x2_dram = [pdram.tile([P, D], f32, tag="x2t", name=f"x2t{_t}") for _t in range(NT)]

        # attention phase scope: qkv operands live through phases A+B only
        ab = ExitStack()
        pqk = ab.enter_context(tc.tile_pool(name="qk", bufs=1))
        pxs = ab.enter_context(tc.tile_pool(name="xs", bufs=2))
        qT = pqk.tile([P, DC, T], bf16, tag="qT")
        kT = pqk.tile([P, DC, T], bf16, tag="kT")
        attnT = pqk.tile([P, DC, T], bf16, tag="attnT")
        v_sb = pqk.tile([P, NT, H, HD + 1], bf16, tag="v")
        nc.vector.memset(v_sb[:, :, :, HD : HD + 1], 1.0)

        # ---------------- Phase A: LN1 + transpose + QKV ----------------
        with tc.tile_pool(name="wq", bufs=1) as pwq, \
             tc.tile_pool(name="xnt", bufs=1) as pxnt, \
             tc.tile_pool(name="pma", bufs=2, space="PSUM") as pma:
            wqkv_sb = pwq.tile([P, DC, 3 * D], bf16)
            for c in range(DC):
                nc.sync.dma_start(wqkv_sb[:, c, :], wqkv_d[P * c : P * (c + 1), :])
            xnT = pxnt.tile([P, DC, T], bf16)

            for t in range(NT):
                xt = pxs.tile([P, D], f32, tag="xs")
                nc.sync.dma_start(xt[:], x_d[P * t : P * (t + 1), :])
                xn = layernorm_tile(xt)
                tp = pma.tile([P, DC, P], bf16, tag="tp")
                for b in range(DC):
                    nc.tensor.transpose(tp[:, b, :], xn[:, P * b : P * (b + 1)], ident[:])
                nc.any.tensor_copy(xnT[:, :, P * t : P * (t + 1)], tp[:])

            for g in range(n_tg):
                gs, ge = QG * g, min(QG * (g + 1), T)
                # q (chunks 0..DC-1) and k (chunks DC..2DC-1), feature-major
                for fc in range(2 * DC):
                    ps = pma.tile([P, QG], f32, tag="mm")
                    for c in range(DC):
                        nc.tensor.matmul(
                            ps[:, : ge - gs],
                            wqkv_sb[:, c, P * fc : P * (fc + 1)],
                            xnT[:, c, gs:ge],
                            start=(c == 0),
                            stop=(c == DC - 1),
                        )
                    if fc < DC:  # q gets its (scaled, folded) bias
                        nc.scalar.add(qT[:, fc, gs:ge], ps[:, : ge - gs], bq_sb[:, fc : fc + 1])
                    else:  # k bias is softmax-invariant: dropped
                        nc.any.tensor_copy(kT[:, fc - DC, gs:ge], ps[:, : ge - gs])
                # v token-major
                for t in range(gs // P, ge // P):
                    for vg in range(n_og):
                        os_, oe = ogs * vg, min(ogs * (vg + 1), D)
                        ps = pma.tile([P, QG], f32, tag="mm")
                        for c in range(DC):
                            nc.tensor.matmul(
                                ps[:, : oe - os_],
                                xnT[:, c, P * t : P * (t + 1)],
                                wqkv_sb[:, c, 2 * D + os_ : 2 * D + oe],
                                start=(c == 0),
                                stop=(c == DC - 1),
                            )
                        nh = (oe - os_) // HD
                        nc.any.tensor_copy(
                            v_sb[:, t, os_ // HD : os_ // HD + nh, 0:HD],
                            ps[:, : oe - os_].rearrange("p (h d) -> p h d", d=HD),
                        )

        # ---------------- Phase B: attention + proj + residual ----------------
        with tc.tile_pool(name="wp", bufs=1) as pwp, \
             tc.tile_pool(name="pt", bufs=2) as ppt, \
             tc.tile_pool(name="rc", bufs=3) as prc, \
             tc.tile_pool(name="x2st", bufs=3) as px2st, \
             tc.tile_pool(name="psc", bufs=2, space="PSUM") as psc, \
             tc.tile_pool(name="ppv", bufs=2, space="PSUM") as ppv, \
             tc.tile_pool(name="pmb", bufs=2, space="PSUM") as pmb:
            wproj_sb = pwp.tile([P, DC, D], bf16)
            for c in range(DC):
                nc.sync.dma_start(wproj_sb[:, c, :], wproj_d[P * c : P * (c + 1), :])

            for toff, nkt in seqs:
                if nkt == 0:
                    continue
                soff = P * toff
                S = P * nkt
                n_qg = _ceil_div(S, QG)
                for h in range(H):
                    hc, hp = h // HPC, HD * (h % HPC)
                    for qg in range(n_qg):
                        qs = soff + QG * qg
                        qe = min(qs + QG, soff + S)
                        qn = qe - qs
                        PT = ppt.tile([P, max_kt, max_qg], bf16, tag="pt")
                        for kp in range(0, nkt, 2):
                            pr = min(2, nkt - kp)
                            sc = psc.tile([P, 2, max_qg], f32, tag="sc")
                            for j in range(pr):
                                kt = kp + j
                                nc.tensor.matmul(
                                    sc[:, j, :qn],
                                    kT[hp : hp + HD, hc, soff + P * kt : soff + P * (kt + 1)],
                                    qT[hp : hp + HD, hc, qs:qe],
                                    start=True,
                                    stop=True,
                                )
                            nc.scalar.activation(
                                PT[:, kp : kp + pr, :qn], sc[:, :pr, :qn], AF.Exp
                            )
                        pv = ppv.tile([HD + 1, max_qg], f32, tag="pv")
                        for kt in range(nkt):
                            nc.tensor.matmul(
                                pv[:, :qn],
                                v_sb[:, toff + kt, h, :],
                                PT[:, kt, :qn],
                                start=(kt == 0),
                                stop=(kt == nkt - 1),
                            )
                        rc = prc.tile([1, max_qg], f32, tag="rc")
                        nc.vector.reciprocal(rc[:, :qn], pv[HD : HD + 1, :qn])
                        rb = prc.tile([HD, max_qg], f32, tag="rb")
                        nc.gpsimd.partition_broadcast(rb[:, :qn], rc[:, :qn])
                        nc.vector.tensor_tensor(
                            attnT[hp : hp + HD, hc, qs:qe],
                            pv[0:HD, :qn],
                            rb[:, :qn],
                            op=OP.mult,
                        )

            # proj + residual 1 (token-major)
            for t in range(NT):
                xt = pxs.tile([P, D], f32, tag="xs")
                nc.sync.dma_start(xt[:], x_d[P * t : P * (t + 1), :])
                x2t = px2st.tile([P, D], f32, tag="x2s")
                for og in range(n_og):
                    os_, oe = ogs * og, min(ogs * (og + 1), D)
                    ps = pmb.tile([P, QG], f32, tag="mm")
                    for c in range(DC):
                        nc.tensor.matmul(
                            ps[:, : oe - os_],
                            attnT[:, c, P * t : P * (t + 1)],
                            wproj_sb[:, c, os_:oe],
                            start=(c == 0),
                            stop=(c == DC - 1),
                        )
                    nc.vector.tensor_tensor(
                        x2t[:, os_:oe], xt[:, os_:oe], ps[:, : oe - os_], op=OP.add
                    )
                if has_row_bias:
                    nc.vector.tensor_tensor(x2t[:], x2t[:], bp_b[:], op=OP.add)
                nc.sync.dma_start(x2_dram[t][:], x2t[:])

        ab.close()  # release qkv/attention SBUF before the MLP weights arrive

        # ---------------- Phase C: MLP + residual ----------------
        with tc.tile_pool(name="w1", bufs=1) as pw1, \
             tc.tile_pool(name="w2", bufs=1) as pw2, \
             tc.tile_pool(name="x2g", bufs=4) as px2g, \
             tc.tile_pool(name="xnt2", bufs=2) as pxnt2, \
             tc.tile_pool(name="g", bufs=3) as pgp, \
             tc.tile_pool(name="yo", bufs=3) as pyo, \
             tc.tile_pool(name="po", bufs=4, space="PSUM") as po, \
             tc.tile_pool(name="pmc", bufs=2, space="PSUM") as pmc:
            wfc1_sb = pw1.tile([P, DC, DF], bf16)
            for r in range(4):
                rs, re = (DF // 4) * r, (DF // 4) * (r + 1)
                for c in range(DC):
                    nc.sync.dma_start(
                        wfc1_sb[:, c, rs:re], wfc1_d[P * c : P * (c + 1), rs:re]
                    )
            wfc2_sb = pw2.tile([P, FC, D], bf16)
            for fch in range(FC):
                nc.sync.dma_start(wfc2_sb[:, fch, :], wfc2_d[P * fch : P * (fch + 1), :])

            for g0 in range(0, NT, 2):
                tiles = [t for t in (g0, g0 + 1) if t < NT]
                ntok = P * len(tiles)
                x2ts = []
                xn2T = pxnt2.tile([P, DC, 2 * P], bf16, tag="xn2T")
                for j, t in enumerate(tiles):
                    x2t = px2g.tile([P, D], f32, tag="x2g")
                    nc.sync.dma_start(x2t[:], x2_dram[t][:])
                    x2ts.append(x2t)
                    xn = layernorm_tile(x2t)
                    tp = pmc.tile([P, DC, P], bf16, tag="tp")
                    for b in range(DC):
                        nc.tensor.transpose(tp[:, b, :], xn[:, P * b : P * (b + 1)], ident[:])
                    nc.any.tensor_copy(xn2T[:, :, P * j : P * (j + 1)], tp[:])

                pots = [
                    [po.tile([P, ogs], f32, tag="po", name=f"po{_j}_{_o}")
                     for _o in range(n_og)]
                    for _j in range(len(tiles))
                ]
                for fch in range(FC):
                    ph = pmc.tile([P, 2 * P], f32, tag="mm")
                    for c in range(DC):
                        nc.tensor.matmul(
                            ph[:, :ntok],
                            wfc1_sb[:, c, P * fch : P * (fch + 1)],
                            xn2T[:, c, :ntok],
                            start=(c == 0),
                            stop=(c == DC - 1),
                        )
                    gt = pgp.tile([P, 2 * P], bf16, tag="g")
                    nc.scalar.activation(
                        gt[:, :ntok], ph[:, :ntok], AF.Gelu, bias=bfc1_sb[:, fch : fch + 1]
                    )
                    for j in range(len(tiles)):
                        for og in range(n_og):
                            os_, oe = ogs * og, min(ogs * (og + 1), D)
                            nc.tensor.matmul(
                                pots[j][og][:, : oe - os_],
                                gt[:, P * j : P * (j + 1)],
                                wfc2_sb[:, fch, os_:oe],
                                start=(fch == 0),
                                stop=(fch == FC - 1),
                            )
                for j, t in enumerate(tiles):
                    yt = pyo.tile([P, D], f32, tag="y")
                    for og in range(n_og):
                        os_, oe = ogs * og, min(ogs * (og + 1), D)
                        nc.vector.tensor_tensor(
                            yt[:, os_:oe], x2ts[j][:, os_:oe], pots[j][og][:, : oe - os_],
                            op=OP.add,
                        )
                    if has_row_bias:
                        nc.vector.tensor_tensor(yt[:], yt[:], bf2_b[:], op=OP.add)
                    nc.sync.dma_start(y_d[P * t : P * (t + 1), :], yt[:])

    nc.compile()
    return nc


def prepare_host(cfg, w_qkv, b_qkv, w_proj, b_proj, ln1_w, ln1_b, ln2_w, ln2_b,
                 w_fc1, b_fc1, w_fc2, b_fc2, gamma1, gamma2):
    """Fold LN affines / score scale / LayerScale into weights. Returns dict of
    device arrays shared by every core, plus has_row_bias flag."""
    D, DF = cfg["D"], cfg["DF"]
    f = np.float32
    w_qkv = np.asarray(w_qkv, f); b_qkv = np.asarray(b_qkv, f)
    w_proj = np.asarray(w_proj, f); b_proj = np.asarray(b_proj, f)
    w_fc1 = np.asarray(w_fc1, f); b_fc1 = np.asarray(b_fc1, f)
    w_fc2 = np.asarray(w_fc2, f); b_fc2 = np.asarray(b_fc2, f)

    # LN1 affine -> qkv;  score scale 1/sqrt(HD) -> q columns
    wq = w_qkv * np.asarray(ln1_w, f)[:, None]
    bq_full = b_qkv + np.asarray(ln1_b, f) @ w_qkv
    scale = 1.0 / np.sqrt(HD)
    wq[:, :D] *= scale
    bq_full = bq_full.copy()
    bq_full[:D] *= scale
    # v bias propagates unchanged through softmax-averaging -> fold into proj bias
    bv = bq_full[2 * D :]
    # gamma1 -> proj
    wp = w_proj * np.asarray(gamma1, f)[None, :]
    bp_row = (b_proj + bv @ w_proj) * np.asarray(gamma1, f)
    # LN2 affine -> fc1
    w1 = w_fc1 * np.asarray(ln2_w, f)[:, None]
    b1 = b_fc1 + np.asarray(ln2_b, f) @ w_fc1
    # gamma2 -> fc2
    w2 = w_fc2 * np.asarray(gamma2, f)[None, :]
    bf2_row = b_fc2 * np.asarray(gamma2, f)

    has_row_bias = bool(np.any(bp_row != 0) or np.any(bf2_row != 0))

    def chunk_cols(b):  # [n*P] -> [P, n] (chunk-major per-partition layout)
        return np.ascontiguousarray(b.reshape(-1, P).T.astype(f))

    shared = {
        "wqkv": wq, "wproj": wp, "wfc1": w1, "wfc2": w2,
        "bq": chunk_cols(bq_full[:D]),
        "bfc1": chunk_cols(b1),
        "ident": np.eye(P, dtype=f),
    }
    if has_row_bias:
        shared["bprow"] = bp_row.reshape(1, D)
        shared["bf2row"] = bf2_row.reshape(1, D)

    import ml_dtypes
    bf = ml_dtypes.bfloat16
    for k in ("wqkv", "wproj", "wfc1", "wfc2", "ident"):
        shared[k] = shared[k].astype(bf)
    return shared, has_row_bias


_BUILD_CACHE = {}


def kernel(x1, x2, w_qkv, b_qkv, w_proj, b_proj, ln1_w, ln1_b, ln2_w, ln2_b,
           w_fc1, b_fc1, w_fc2, b_fc2, gamma1, gamma2):
    from concourse import bass_utils

    cfg = FULL_CFG
    x1 = np.asarray(x1, np.float32)
    x2 = np.asarray(x2, np.float32)
    B = x1.shape[0]
    assert B == 8 and x2.shape[0] == 8

    shared, has_row_bias = prepare_host(
        cfg, w_qkv, b_qkv, w_proj, b_proj, ln1_w, ln1_b, ln2_w, ln2_b,
        w_fc1, b_fc1, w_fc2, b_fc2, gamma1, gamma2)

    key = (tuple(sorted(cfg.items())), has_row_bias)
    if key not in _BUILD_CACHE:
        _BUILD_CACHE[key] = build_bass(cfg, has_row_bias=has_row_bias)
    nc = _BUILD_CACHE[key]

    in_maps = []
    for c in range(B):
        m = dict(shared)
        m["x"] = np.ascontiguousarray(
            np.concatenate([x1[c], x2[c]], axis=0), dtype=np.float32)
        in_maps.append(m)

    res = bass_utils.run_bass_kernel_spmd(nc, in_maps, core_ids=list(range(B)))
    SA = cfg["SA"]
    y1 = np.stack([res.results[c]["y"][:SA] for c in range(B)])
    y2 = np.stack([res.results[c]["y"][SA:] for c in range(B)])
    return (y1, y2)


# revision 11
# speedup vs baseline: 1.0112x; 1.0112x over previous
"""Trainium2 Bass kernel: NestedTensorBlock (ViT block w/ LayerScale) over two
ragged groups  x1:[8,1024,1024]  x2:[8,512,1024],  D=1024, H=16 heads.

Sharding: pure data parallel — core c gets sequence c of each group
(1024 + 512 = 1536 tokens per core), no collectives.

Numerics: residual stream fp32; all GEMMs stream bf16 with fp32 PSUM
accumulation.  LayerScale gamma (1e-5) damps the attention/MLP branches, so
bf16 branch error is invisible in the output; host-side we fold LN affine,
the 1/sqrt(hd) score scale, and gamma into the weights.

Layout strategy per core (avoids all transposes except LN outputs):
  - LN token-major -> PE-transpose to xn^T [D, T]
  - q^T, k^T feature-major [D, T];  v token-major [T, H, 64+1] with a ones
    column (PV matmul then also emits the softmax denominator for free)
  - scores^T [k, q] straight from PE; exp on ACT with no max subtraction
    (scores are provably < ~3 for this problem's scales); P^T streams into
    the PV matmul with no transpose of probabilities
  - attn^T [D, T] feature-major feeds proj; proj/fc2 emit token-major for
    fp32 residual adds; fc1 emits feature-major h^T for gelu + fc2.
"""

import numpy as np

P = 128
HD = 64
LN_EPS = 1e-5
QG = 512  # token-group size for matmul free dims

FULL_CFG = dict(D=1024, H=16, SA=1024, SB=512, DF=4096)


def _ceil_div(a, b):
    return (a + b - 1) // b


def build_bass(cfg, has_row_bias=False):
    """Builds and compiles the per-core Bass program. Returns nc."""
    import concourse.bacc as bacc
    import concourse.tile as tile
    from concourse import mybir
    from contextlib import ExitStack

    f32 = mybir.dt.float32
    bf16 = mybir.dt.bfloat16
    AF = mybir.ActivationFunctionType
    OP = mybir.AluOpType

    D, H, SA, SB, DF = cfg["D"], cfg["H"], cfg["SA"], cfg["SB"], cfg["DF"]
    T = SA + SB
    NT = T // P                     # token tiles
    DC = D // P                     # D chunks (contraction chunks)
    FC = DF // P                    # fc1 output chunks
    HPC = P // HD                   # heads per feature chunk (2)
    n_og = _ceil_div(D, QG)         # output col groups for proj/fc2
    ogs = min(QG, D)
    n_tg = _ceil_div(T, QG)         # token groups for qkv
    seqs = [(0, SA // P), (SA // P, SB // P)]  # (tile offset, n tiles)
    max_kt = max(n for _, n in seqs)
    max_qg = min(QG, max(SA, SB))
    n_stat = _ceil_div(D, 512)

    nc = bacc.Bacc("TRN2", target_bir_lowering=False, debug=False)

    x_d = nc.dram_tensor("x", [T, D], f32, kind="ExternalInput").ap()
    wqkv_d = nc.dram_tensor("wqkv", [D, 3 * D], bf16, kind="ExternalInput").ap()
    wproj_d = nc.dram_tensor("wproj", [D, D], bf16, kind="ExternalInput").ap()
    wfc1_d = nc.dram_tensor("wfc1", [D, DF], bf16, kind="ExternalInput").ap()
    wfc2_d = nc.dram_tensor("wfc2", [DF, D], bf16, kind="ExternalInput").ap()
    ident_d = nc.dram_tensor("ident", [P, P], bf16, kind="ExternalInput").ap()
    bq_d = nc.dram_tensor("bq", [P, DC], f32, kind="ExternalInput").ap()
    bfc1_d = nc.dram_tensor("bfc1", [P, FC], f32, kind="ExternalInput").ap()
    if has_row_bias:
        bp_d = nc.dram_tensor("bprow", [1, D], f32, kind="ExternalInput").ap()
        bf2_d = nc.dram_tensor("bf2row", [1, D], f32, kind="ExternalInput").ap()
    y_d = nc.dram_tensor("y", [T, D], f32, kind="ExternalOutput").ap()

    with tile.TileContext(nc) as tc, ExitStack() as ctx:
        const = ctx.enter_context(tc.tile_pool(name="const", bufs=1))
        ident = const.tile([P, P], bf16)
        nc.sync.dma_start(ident[:], ident_d)
        bq_sb = const.tile([P, DC], f32)
        nc.sync.dma_start(bq_sb[:], bq_d)
        bfc1_sb = const.tile([P, FC], f32)
        nc.sync.dma_start(bfc1_sb[:], bfc1_d)
        eps_sb = const.tile([P, 1], f32)
        nc.vector.memset(eps_sb[:], LN_EPS)
        if has_row_bias:
            bp_row = const.tile([1, D], f32)
            nc.sync.dma_start(bp_row[:], bp_d)
            bp_b = const.tile([P, D], f32)
            nc.gpsimd.partition_broadcast(bp_b[:], bp_row[:])
            bf2_row = const.tile([1, D], f32)
            nc.sync.dma_start(bf2_row[:], bf2_d)
            bf2_b = const.tile([P, D], f32)
            nc.gpsimd.partition_broadcast(bf2_b[:], bf2_row[:])

        # small pools shared by both layernorms
        pst = ctx.enter_context(tc.tile_pool(name="st", bufs=4))
        pxn = ctx.enter_context(tc.tile_pool(name="xn", bufs=2))

        def layernorm_tile(xt):
            """xt: [P, D] f32 SBUF -> returns [P, D] bf16 standardized."""
            stats = pst.tile([P, n_stat, 6], f32, tag="stats")
            for i in range(n_stat):
                lo, hi = 512 * i, min(512 * (i + 1), D)
                nc.vector.bn_stats(stats[:, i, :], xt[:, lo:hi])
            mv = pst.tile([P, 2], f32, tag="mv")
            nc.vector.bn_aggr(mv[:], stats[:])
            std = pst.tile([P, 1], f32, tag="std")
            nc.scalar.activation(std[:], mv[:, 1:2], AF.Sqrt, bias=eps_sb[:])
            rstd = pst.tile([P, 1], f32, tag="rstd")
            nc.vector.reciprocal(rstd[:], std[:])
            xn = pxn.tile([P, D], bf16, tag="xn")
            nc.vector.tensor_scalar(
                xn[:], xt[:], mv[:, 0:1], rstd[:], op0=OP.subtract, op1=OP.mult
            )
            return xn

        # per-tile fp32 x2 scratch in DRAM (residual stream between halves)
        pdram = ctx.enter_context(tc.tile_pool(name="x2d", bufs=NT, space="DRAM"))
        x2_dram = [pdram.tile([P, D], f32, tag="x2t", name=f"x2t{_t}") for _t in range(NT)]

        # attention phase scope: qkv operands live through phases A+B only
        ab = ExitStack()
        pqk = ab.enter_context(tc.tile_pool(name="qk", bufs=1))
        pxs = ab.enter_context(tc.tile_pool(name="xs", bufs=2))
        qT = pqk.tile([P, DC, T], bf16, tag="qT")
        kT = pqk.tile([P, DC, T], bf16, tag="kT")
        attnT = pqk.tile([P, DC, T], bf16, tag="attnT")
        v_sb = pqk.tile([P, NT, H, HD + 1], bf16, tag="v")
        nc.vector.memset(v_sb[:, :, :, HD : HD + 1], 1.0)

        # ---------------- Phase A: LN1 + transpose + QKV ----------------
        with tc.tile_pool(name="wq", bufs=1) as pwq, \
             tc.tile_pool(name="xnt", bufs=1) as pxnt, \
             tc.tile_pool(name="pma", bufs=2, space="PSUM") as pma:
            wqkv_sb = pwq.tile([P, DC, 3 * D], bf16)
            for c in range(DC):
                nc.sync.dma_start(wqkv_sb[:, c, :], wqkv_d[P * c : P * (c + 1), :])
            xnT = pxnt.tile([P, DC, T], bf16)

            for t in range(NT):
                xt = pxs.tile([P, D], f32, tag="xs")
                nc.sync.dma_start(xt[:], x_d[P * t : P * (t + 1), :])
                xn = layernorm_tile(xt)
                tp = pma.tile([P, DC, P], bf16, tag="tp")
                for b in range(DC):
                    nc.tensor.transpose(tp[:, b, :], xn[:, P * b : P * (b + 1)], ident[:])
                nc.any.tensor_copy(xnT[:, :, P * t : P * (t + 1)], tp[:])

            for g in range(n_tg):
                gs, ge = QG * g, min(QG * (g + 1), T)
                # q (chunks 0..DC-1) and k (chunks DC..2DC-1), feature-major
                for fc in range(2 * DC):
                    ps = pma.tile([P, QG], f32, tag="mm")
                    for c in range(DC):
                        nc.tensor.matmul(
                            ps[:, : ge - gs],
                            wqkv_sb[:, c, P * fc : P * (fc + 1)],
                            xnT[:, c, gs:ge],
                            start=(c == 0),
                            stop=(c == DC - 1),
                        )
                    if fc < DC:  # q gets its (scaled, folded) bias
                        nc.scalar.add(qT[:, fc, gs:ge], ps[:, : ge - gs], bq_sb[:, fc : fc + 1])
                    else:  # k bias is softmax-invariant: dropped
                        nc.vector.tensor_copy(kT[:, fc - DC, gs:ge], ps[:, : ge - gs])
                # v token-major
                for t in range(gs // P, ge // P):
                    for vg in range(n_og):
                        os_, oe = ogs * vg, min(ogs * (vg + 1), D)
                        ps = pma.tile([P, QG], f32, tag="mm")
                        for c in range(DC):
                            nc.tensor.matmul(
                                ps[:, : oe - os_],
                                xnT[:, c, P * t : P * (t + 1)],
                                wqkv_sb[:, c, 2 * D + os_ : 2 * D + oe],
                                start=(c == 0),
                                stop=(c == DC - 1),
                            )
                        nh = (oe - os_) // HD
                        nc.vector.tensor_copy(
                            v_sb[:, t, os_ // HD : os_ // HD + nh, 0:HD],
                            ps[:, : oe - os_].rearrange("p (h d) -> p h d", d=HD),
                        )

        # ---------------- Phase B: attention + proj + residual ----------------
        with tc.tile_pool(name="wp", bufs=1) as pwp, \
             tc.tile_pool(name="pt", bufs=2) as ppt, \
             tc.tile_pool(name="rc", bufs=2) as prc, \
             tc.tile_pool(name="rb", bufs=3) as prb, \
             tc.tile_pool(name="x2st", bufs=3) as px2st, \
             tc.tile_pool(name="psc", bufs=2, space="PSUM") as psc, \
             tc.tile_pool(name="ppv", bufs=2, space="PSUM") as ppv, \
             tc.tile_pool(name="pmb", bufs=2, space="PSUM") as pmb:
            wproj_sb = pwp.tile([P, DC, D], bf16)
            for c in range(DC):
                nc.sync.dma_start(wproj_sb[:, c, :], wproj_d[P * c : P * (c + 1), :])

            for toff, nkt in seqs:
                if nkt == 0:
                    continue
                soff = P * toff
                S = P * nkt
                n_qg = _ceil_div(S, QG)
                for qg in range(n_qg):
                    qs = soff + QG * qg
                    qe = min(qs + QG, soff + S)
                    qn = qe - qs
                    for h in range(H):
                        hc, hp = h // HPC, HD * (h % HPC)
                        PT = ppt.tile([P, max_kt, max_qg], bf16, tag="pt")
                        for kp in range(0, nkt, 2):
                            pr = min(2, nkt - kp)
                            sc = psc.tile([P, 2, max_qg], f32, tag="sc")
                            for j in range(pr):
                                kt = kp + j
                                nc.tensor.matmul(
                                    sc[:, j, :qn],
                                    kT[hp : hp + HD, hc, soff + P * kt : soff + P * (kt + 1)],
                                    qT[hp : hp + HD, hc, qs:qe],
                                    start=True,
                                    stop=True,
                                )
                            nc.scalar.activation(
                                PT[:, kp : kp + pr, :qn], sc[:, :pr, :qn], AF.Exp
                            )
                        pv = ppv.tile([HD + 1, max_qg], f32, tag="pv")
                        for kt in range(nkt):
                            nc.tensor.matmul(
                                pv[:, :qn],
                                v_sb[:, toff + kt, h, :],
                                PT[:, kt, :qn],
                                start=(kt == 0),
                                stop=(kt == nkt - 1),
                            )
                        rc = prc.tile([1, max_qg], f32, tag="rc")
                        nc.vector.reciprocal(rc[:, :qn], pv[HD : HD + 1, :qn])
                        rb = prb.tile([P, max_qg], f32, tag="rb")
                        nc.gpsimd.partition_broadcast(rb[:, :qn], rc[:, :qn])
                        nc.vector.tensor_tensor(
                            attnT[hp : hp + HD, hc, qs:qe],
                            pv[0:HD, :qn],
                            rb[hp : hp + HD, :qn],
                            op=OP.mult,
                        )

            # proj + residual 1 (token-major)
            for t in range(NT):
                xt = pxs.tile([P, D], f32, tag="xs")
                nc.sync.dma_start(xt[:], x_d[P * t : P * (t + 1), :])
                x2t = px2st.tile([P, D], f32, tag="x2s")
                for og in range(n_og):
                    os_, oe = ogs * og, min(ogs * (og + 1), D)
                    ps = pmb.tile([P, QG], f32, tag="mm")
                    for c in range(DC):
                        nc.tensor.matmul(
                            ps[:, : oe - os_],
                            attnT[:, c, P * t : P * (t + 1)],
                            wproj_sb[:, c, os_:oe],
                            start=(c == 0),
                            stop=(c == DC - 1),
                        )
                    nc.vector.tensor_tensor(
                        x2t[:, os_:oe], xt[:, os_:oe], ps[:, : oe - os_], op=OP.add
                    )
                if has_row_bias:
                    nc.vector.tensor_tensor(x2t[:], x2t[:], bp_b[:], op=OP.add)
                nc.sync.dma_start(x2_dram[t][:], x2t[:])

        ab.close()  # release qkv/attention SBUF before the MLP weights arrive

        # ---------------- Phase C: MLP + residual ----------------
        with tc.tile_pool(name="w1", bufs=1) as pw1, \
             tc.tile_pool(name="w2", bufs=1) as pw2, \
             tc.tile_pool(name="x2g", bufs=4) as px2g, \
             tc.tile_pool(name="xnt2", bufs=2) as pxnt2, \
             tc.tile_pool(name="g", bufs=3) as pgp, \
             tc.tile_pool(name="yo", bufs=3) as pyo, \
             tc.tile_pool(name="po", bufs=4, space="PSUM") as po, \
             tc.tile_pool(name="pmc", bufs=2, space="PSUM") as pmc:
            wfc1_sb = pw1.tile([P, DC, DF], bf16)
            for r in range(4):
                rs, re = (DF // 4) * r, (DF // 4) * (r + 1)
                for c in range(DC):
                    nc.sync.dma_start(
                        wfc1_sb[:, c, rs:re], wfc1_d[P * c : P * (c + 1), rs:re]
                    )
            wfc2_sb = pw2.tile([P, FC, D], bf16)
            for fch in range(FC):
                nc.sync.dma_start(wfc2_sb[:, fch, :], wfc2_d[P * fch : P * (fch + 1), :])

            for g0 in range(0, NT, 2):
                tiles = [t for t in (g0, g0 + 1) if t < NT]
                ntok = P * len(tiles)
                x2ts = []
                xn2T = pxnt2.tile([P, DC, 2 * P], bf16, tag="xn2T")
                for j, t in enumerate(tiles):
                    x2t = px2g.tile([P, D], f32, tag="x2g")
                    nc.sync.dma_start(x2t[:], x2_dram[t][:])
                    x2ts.append(x2t)
                    xn = layernorm_tile(x2t)
                    tp = pmc.tile([P, DC, P], bf16, tag="tp")
                    for b in range(DC):
                        nc.tensor.transpose(tp[:, b, :], xn[:, P * b : P * (b + 1)], ident[:])
                    nc.any.tensor_copy(xn2T[:, :, P * j : P * (j + 1)], tp[:])

                pots = [
                    [po.tile([P, ogs], f32, tag="po", name=f"po{_j}_{_o}")
                     for _o in range(n_og)]
                    for _j in range(len(tiles))
                ]
                for fch in range(FC):
                    ph = pmc.tile([P, 2 * P], f32, tag="mm")
                    for c in range(DC):
                        nc.tensor.matmul(
                            ph[:, :ntok],
                            wfc1_sb[:, c, P * fch : P * (fch + 1)],
                            xn2T[:, c, :ntok],
                            start=(c == 0),
                            stop=(c == DC - 1),
                        )
                    gt = pgp.tile([P, 2 * P], bf16, tag="g")
                    nc.scalar.activation(
                        gt[:, :ntok], ph[:, :ntok], AF.Gelu, bias=bfc1_sb[:, fch : fch + 1]
                    )
                    for j in range(len(tiles)):
                        for og in range(n_og):
                            os_, oe = ogs * og, min(ogs * (og + 1), D)
                            nc.tensor.matmul(
                                pots[j][og][:, : oe - os_],
                                gt[:, P * j : P * (j + 1)],
                                wfc2_sb[:, fch, os_:oe],
                                start=(fch == 0),
                                stop=(fch == FC - 1),
                            )
                for j, t in enumerate(tiles):
                    yt = pyo.tile([P, D], f32, tag="y")
                    for og in range(n_og):
                        os_, oe = ogs * og, min(ogs * (og + 1), D)
                        nc.vector.tensor_tensor(
                            yt[:, os_:oe], x2ts[j][:, os_:oe], pots[j][og][:, : oe - os_],
                            op=OP.add,
                        )
                    if has_row_bias:
                        nc.vector.tensor_tensor(yt[:], yt[:], bf2_b[:], op=OP.add)
                    nc.sync.dma_start(y_d[P * t : P * (t + 1), :], yt[:])

    nc.compile()
    return nc


def prepare_host(cfg, w_qkv, b_qkv, w_proj, b_proj, ln1_w, ln1_b, ln2_w, ln2_b,
                 w_fc1, b_fc1, w_fc2, b_fc2, gamma1, gamma2):
    """Fold LN affines / score scale / LayerScale into weights. Returns dict of
    device arrays shared by every core, plus has_row_bias flag."""
    D, DF = cfg["D"], cfg["DF"]
    f = np.float32
    w_qkv = np.asarray(w_qkv, f); b_qkv = np.asarray(b_qkv, f)
    w_proj = np.asarray(w_proj, f); b_proj = np.asarray(b_proj, f)
    w_fc1 = np.asarray(w_fc1, f); b_fc1 = np.asarray(b_fc1, f)
    w_fc2 = np.asarray(w_fc2, f); b_fc2 = np.asarray(b_fc2, f)

    # LN1 affine -> qkv;  score scale 1/sqrt(HD) -> q columns
    wq = w_qkv * np.asarray(ln1_w, f)[:, None]
    bq_full = b_qkv + np.asarray(ln1_b, f) @ w_qkv
    scale = 1.0 / np.sqrt(HD)
    wq[:, :D] *= scale
    bq_full = bq_full.copy()
    bq_full[:D] *= scale
    # v bias propagates unchanged through softmax-averaging -> fold into proj bias
    bv = bq_full[2 * D :]
    # gamma1 -> proj
    wp = w_proj * np.asarray(gamma1, f)[None, :]
    bp_row = (b_proj + bv @ w_proj) * np.asarray(gamma1, f)
    # LN2 affine -> fc1
    w1 = w_fc1 * np.asarray(ln2_w, f)[:, None]
    b1 = b_fc1 + np.asarray(ln2_b, f) @ w_fc1
    # gamma2 -> fc2
    w2 = w_fc2 * np.asarray(gamma2, f)[None, :]
    bf2_row = b_fc2 * np.asarray(gamma2, f)

    has_row_bias = bool(np.any(bp_row != 0) or np.any(bf2_row != 0))

    def chunk_cols(b):  # [n*P] -> [P, n] (chunk-major per-partition layout)
        return np.ascontiguousarray(b.reshape(-1, P).T.astype(f))

    shared = {
        "wqkv": wq, "wproj": wp, "wfc1": w1, "wfc2": w2,
        "bq": chunk_cols(bq_full[:D]),
        "bfc1": chunk_cols(b1),
        "ident": np.eye(P, dtype=f),
    }
    if has_row_bias:
        shared["bprow"] = bp_row.reshape(1, D)
        shared["bf2row"] = bf2_row.reshape(1, D)

    import ml_dtypes
    bf = ml_dtypes.bfloat16
    for k in ("wqkv", "wproj", "wfc1", "wfc2", "ident"):
        shared[k] = shared[k].astype(bf)
    return shared, has_row_bias


_BUILD_CACHE = {}


def kernel(x1, x2, w_qkv, b_qkv, w_proj, b_proj, ln1_w, ln1_b, ln2_w, ln2_b,
           w_fc1, b_fc1, w_fc2, b_fc2, gamma1, gamma2):
    from concourse import bass_utils

    cfg = FULL_CFG
    x1 = np.asarray(x1, np.float32)
    x2 = np.asarray(x2, np.float32)
    B = x1.shape[0]
    assert B == 8 and x2.shape[0] == 8

    shared, has_row_bias = prepare_host(
        cfg, w_qkv, b_qkv, w_proj, b_proj, ln1_w, ln1_b, ln2_w, ln2_b,
        w_fc1, b_fc1, w_fc2, b_fc2, gamma1, gamma2)

    key = (tuple(sorted(cfg.items())), has_row_bias)
    if key not in _BUILD_CACHE:
        _BUILD_CACHE[key] = build_bass(cfg, has_row_bias=has_row_bias)
    nc = _BUILD_CACHE[key]

    in_maps = []
    for c in range(B):
        m = dict(shared)
        m["x"] = np.ascontiguousarray(
            np.concatenate([x1[c], x2[c]], axis=0), dtype=np.float32)
        in_maps.append(m)

    res = bass_utils.run_bass_kernel_spmd(nc, in_maps, core_ids=list(range(B)))
    SA = cfg["SA"]
    y1 = np.stack([res.results[c]["y"][:SA] for c in range(B)])
    y2 = np.stack([res.results[c]["y"][SA:] for c in range(B)])
    return (y1, y2)
